# revision 8
# baseline (speedup 1.0000x reference)
"""DINOv2 self-attention (QKV projection + SDPA, no out-proj) on 8 Trainium2
NeuronCores.

Sharding: pure data-parallel over batch (B=8 -> one batch element per core);
no cross-core communication.

v2 design (vs the f32r baseline):
  * All matmul operands in fp16 (f32 PSUM accumulation).  Halves SBUF so
    xT stays resident for the whole pass, enabling QKV projections to be
    interleaved into the attention stream.  The exp stream on ACT
    (~250us busy) is the bottleneck engine; everything else hides under it.
  * ctx matmuls (probs @ v) are split into 64-row contraction halves so two
    heads' MMs run concurrently in disjoint PE row-groups (tile_position
    auto-derived from base partitions), halving effective ctx time, same
    trick the baseline used for scores.
  * QKV is computed just-in-time: prologue = k/q head-pair 0 + first
    quarter of v; the remaining projection work is drained 2-5 ops per
    kt-iteration inside the attention loop so PE gaps under the ACT-bound
    exp stream are filled.
  * Softmax denominator fused into ctx via a ones-column per head
    (v_ext[:, t, h*65+64] = 1); bias of v folded in the normal v path;
    scores' exp fused with the 1/sqrt(hd) scale on ACT.

Per-core engine budget (S=1370, D=1024, H=16, hd=64):
  ACT exp: 264 instrs x ~1.04us = ~250us  <- critical path
  PE: qkv 263k cyc + scores 120k (paired) + ctx 120k (paired) + transposes
      ~= 215us, hidden under ACT except the ~20us prologue.
"""

import numpy as np
from contextlib import ExitStack

import concourse.bass as bass
import concourse.bacc as bacc
import concourse.tile as tile
from concourse import mybir
from concourse import bass_utils
from concourse.masks import make_identity

S, D, H, HD = 1370, 1024, 16, 64
F32 = mybir.dt.float32
F16 = mybir.dt.float16
ND = D // 128                      # 8 contraction tiles
NO = D // 128                      # 8 output tiles per projection
NT = (S + 127) // 128              # 11 token tiles
TSZ = [min(128, S - i * 128) for i in range(NT)]
CHUNKS = [(0, 512), (512, 512), (1024, S - 1024)]
EXP = mybir.ActivationFunctionType.Exp
# ops drained from the deferred projection queue per kt-iteration, per hp
DRAIN = [6, 5, 4, 4, 3, 2, 2, 2]


def _body(tc, xT, wT, bT, bvb, out, reps=1):
    nc = tc.nc
    with ExitStack() as ctx:
        const = ctx.enter_context(tc.tile_pool(name="const", bufs=1))
        ident = const.tile([65, 65], F16)
        make_identity(nc, ident)
        bT_sb = const.tile([128, 24], F32)
        nc.sync.dma_start(bT_sb[:], bT[:])
        bvb_sb = const.tile([128, D], F32)
        nc.scalar.dma_start(bvb_sb[:], bvb[:])
        big = ctx.enter_context(tc.tile_pool(name="big", bufs=1))
        for _rep in range(reps):
            _one_pass(tc, big, ident, bT_sb, bvb_sb, xT, wT, out)


def _one_pass(tc, big, ident, bT_sb, bvb_sb, xT, wT, out):
    nc = tc.nc
    qT = big.tile([128, NO, S], F16, tag="qT", name="qT")
    kT = big.tile([128, NO, S], F16, tag="kT", name="kT")
    v_ext = big.tile([128, NT, H * 65], F16, tag="vext", name="v_ext")
    xt = big.tile([128, ND, S], F16, tag="xt", name="xt")

    with ExitStack() as s:
        wv_pool = s.enter_context(tc.tile_pool(name="wv", bufs=2))
        wqk_pool = s.enter_context(tc.tile_pool(name="wqk", bufs=3))
        et_pool = s.enter_context(tc.tile_pool(name="et", bufs=5))
        cs_pool = s.enter_context(tc.tile_pool(name="cs", bufs=4))
        os_pool = s.enter_context(tc.tile_pool(name="os", bufs=8))
        rec_pool = s.enter_context(tc.tile_pool(name="rec", bufs=8))
        pss = s.enter_context(tc.tile_pool(name="pss", bufs=2, space="PSUM"))
        psc = s.enter_context(tc.tile_pool(name="psc", bufs=1, space="PSUM"))
        psp = s.enter_context(tc.tile_pool(name="psp", bufs=2, space="PSUM"))

        # ones columns (h*65+64) for the fused softmax denominator
        for t in range(NT):
            ones_view = v_ext[:, t, :].rearrange("p (h e) -> p h e", e=65)[:, :, 64]
            nc.vector.tensor_scalar(
                ones_view, bT_sb[:, 0:16],
                0.0, 1.0, mybir.AluOpType.mult, mybir.AluOpType.add)

        # x loads, chunk-outer so chunk-0 compute starts early
        for (c0, cw) in CHUNKS:
            for d in range(ND):
                nc.sync.dma_start(xt[:, d, c0:c0 + cw],
                                  xT[d * 128:(d + 1) * 128, c0:c0 + cw])

        # ---- deferred projection work (generators yielding small ops) ----

        def gen_v_quarter(q):
            """v columns q*256:(q+1)*256 (heads 4q..4q+3), scattered into
            v_ext with the ones-columns skipped."""
            wv = wv_pool.tile([128, ND, 256], F16, tag="wv", name=f"wv{q}")
            cq = 2 * D + q * 256

            def dma():
                for d in range(ND):
                    nc.scalar.dma_start(
                        wv[:, d, :], wT[d * 128:(d + 1) * 128, cq:cq + 256])
            yield dma
            for t in range(NT):
                tsz = TSZ[t]
                ps = psp.tile([128, 512], F32, tag="psp", name="psv")
                for d in range(ND):
                    def mm(t=t, d=d, ps=ps, tsz=tsz):
                        nc.tensor.matmul(
                            ps[:tsz, :256], xt[:, d, t * 128:t * 128 + tsz],
                            wv[:, d, :], start=(d == 0), stop=(d == ND - 1))
                    yield mm

                def evac(t=t, ps=ps, tsz=tsz):
                    dst = v_ext[:tsz, t, :].rearrange(
                        "p (h e) -> p h e", e=65)[:, 4 * q:4 * q + 4, 0:64]
                    src = ps[:tsz, :256].rearrange("p (h e) -> p h e", e=64)
                    bias = bvb_sb[:tsz, q * 256:(q + 1) * 256].rearrange(
                        "p (h e) -> p h e", e=64)
                    nc.vector.tensor_add(dst, src, bias)
                yield evac

        def gen_qk_o(o, w=None, projs=(1, 0), chunks=(0, 1, 2)):
            """q/k projections for o-tile o (heads 2o, 2o+1)."""
            if w is None:
                w = wqk_pool.tile([128, 2, ND, 128], F16, tag="wqk",
                                  name=f"wqk{o}")

                def dma():
                    for proj in (0, 1):
                        for d in range(ND):
                            c = proj * D + o * 128
                            nc.scalar.dma_start(
                                w[:, proj, d, :],
                                wT[d * 128:(d + 1) * 128, c:c + 128])
                yield dma
            for proj in projs:
                dstT = qT if proj == 0 else kT
                for ci in chunks:
                    c0, cw = CHUNKS[ci]
                    ps = psp.tile([128, 512], F32, tag="psp", name="psqk")
                    for d in range(ND):
                        def mm(proj=proj, d=d, ps=ps, c0=c0, cw=cw):
                            nc.tensor.matmul(
                                ps[:, :cw], w[:, proj, d, :],
                                xt[:, d, c0:c0 + cw],
                                start=(d == 0), stop=(d == ND - 1))
                        yield mm

                    def evac(proj=proj, ps=ps, c0=c0, cw=cw, o=o):
                        nc.vector.tensor_scalar_add(
                            dstT[:, o, c0:c0 + cw], ps[:, :cw],
                            bT_sb[:, proj * 8 + o:proj * 8 + o + 1])
                    yield evac

        # ---- prologue: k/q for head-pair 0, v quarter 0 ----
        w0 = wqk_pool.tile([128, 2, ND, 128], F16, tag="wqk", name="wqk0")
        for proj in (0, 1):
            for d in range(ND):
                c = proj * D
                nc.scalar.dma_start(w0[:, proj, d, :],
                                    wT[d * 128:(d + 1) * 128, c:c + 128])
        for op in gen_qk_o(0, w=w0, projs=(1,), chunks=(0,)):
            op()
        for op in gen_qk_o(0, w=w0, projs=(0,), chunks=(0,)):
            op()
        for op in gen_v_quarter(0):          # heads 0-3, fully in prologue
            op()
        for op in gen_qk_o(0, w=w0, projs=(1,), chunks=(1, 2)):
            op()

        # Deferred projection work, drained a few ops per kt-iteration so PE
        # gaps under the ACT-bound exp stream are filled.  finish(name) is a
        # hard barrier: everything up to and including that generator is
        # EMITTED before the first instruction that reads its outputs (Tile
        # deps follow emission order, so this is a correctness requirement,
        # not just a performance one).
        class Work:
            def __init__(self, items):
                self.items = list(items)
                self.idx = 0

            def drain(self, n):
                while n > 0 and self.idx < len(self.items):
                    op = next(self.items[self.idx][1], None)
                    if op is None:
                        self.idx += 1
                        continue
                    op()
                    n -= 1

            def finish(self, name):
                while self.idx < len(self.items):
                    nm, g = self.items[self.idx]
                    for op in g:
                        op()
                    self.idx += 1
                    if nm == name:
                        return

        work = Work([
            ("q0c1", gen_qk_o(0, w=w0, projs=(0,), chunks=(1,))),
            ("q0c2", gen_qk_o(0, w=w0, projs=(0,), chunks=(2,))),
            ("o1", gen_qk_o(1)),
            ("o2", gen_qk_o(2)),
            ("vq1", gen_v_quarter(1)),               # heads 4-7, by hp2
            ("o3", gen_qk_o(3)),
            ("o4", gen_qk_o(4)),
            ("vq2", gen_v_quarter(2)),               # heads 8-11, by hp4
            ("o5", gen_qk_o(5)),
            ("o6", gen_qk_o(6)),
            ("vq3", gen_v_quarter(3)),               # heads 12-15, by hp6
            ("o7", gen_qk_o(7)),
        ])
        BARRIER = {1: "o1", 2: "vq1", 3: "o3", 4: "vq2", 5: "o5",
                   6: "vq3", 7: "o7"}

        def drain(n):
            work.drain(n)

        # ---- attention, head-pair outer ----

        def emit_ctx(pcs, ets, hp, kt, cw):
            ksz = TSZ[kt]
            et = ets.pop(kt)
            for hi in range(2):
                h = 2 * hp + hi
                nc.tensor.matmul(
                    pcs[:, hi, :cw],
                    v_ext[:ksz, kt, h * 65:(h + 1) * 65],
                    et[:ksz, hi, :cw],
                    start=(kt == 0), stop=(kt == NT - 1))

        def flush(hp, c0, cw, csts):
            sub = [(s0, min(128, cw - s0)) for s0 in range(0, cw, 128)]
            oss = [os_pool.tile([128, 128], F32, tag="os", name="os")
                   for _ in sub]
            for hi, cst in enumerate(csts):
                for si, (s0, ssz) in enumerate(sub):
                    tp = psp.tile([128, 65], F16, tag="psp", name="tp")
                    nc.tensor.transpose(
                        tp[:ssz, :], cst[:65, s0:s0 + ssz], ident[:65, :65])
                    rec = rec_pool.tile([128, 1], F32, tag="rec", name="rec")
                    nc.vector.reciprocal(rec[:ssz], tp[:ssz, 64:65])
                    nc.vector.tensor_scalar_mul(
                        oss[si][:ssz, hi * 64:(hi + 1) * 64],
                        tp[:ssz, 0:64], rec[:ssz])
            for si, (s0, ssz) in enumerate(sub):
                nc.sync.dma_start(
                    out[c0 + s0:c0 + s0 + ssz, hp * 128:(hp + 1) * 128],
                    oss[si][:ssz, :])

        for hp in range(8):
            if hp in BARRIER:
                work.finish(BARRIER[hp])
            for ci, (c0, cw) in enumerate(CHUNKS):
                if hp == 0 and ci >= 1:
                    work.finish(f"q0c{ci}")
                pcs = psc.tile([65, 2, 512], F32, tag="psc", name="psc")
                ets = {}
                for kt in range(NT):
                    k0, ksz = kt * 128, TSZ[kt]
                    ps_s = pss.tile([128, 2, 512], F32, tag="pss", name="pss")
                    for hi in range(2):
                        p0 = hi * 64
                        nc.tensor.matmul(
                            ps_s[:ksz, hi, :cw],
                            kT[p0:p0 + 64, hp, k0:k0 + ksz],
                            qT[p0:p0 + 64, hp, c0:c0 + cw],
                            start=True, stop=True)
                    if kt >= 1:
                        emit_ctx(pcs, ets, hp, kt - 1, cw)
                    et = et_pool.tile([128, 2, 512], F16, tag="et", name="et")
                    ets[kt] = et
                    nc.scalar.activation(
                        et[:ksz, :, :cw], ps_s[:ksz, :, :cw], EXP, scale=0.125)
                    drain(DRAIN[hp])
                emit_ctx(pcs, ets, hp, NT - 1, cw)
                csts = []
                for hi in range(2):
                    cst = cs_pool.tile([65, 512], F16, tag="cs", name="cs")
                    nc.vector.tensor_copy(cst[:, :cw], pcs[:, hi, :cw])
                    csts.append(cst)
                flush(hp, c0, cw, csts)
        drain(10**9)


def build_program(reps=1):
    nc = bacc.Bacc("TRN2", target_bir_lowering=False, debug=False,
                   num_devices=8)
    xT = nc.dram_tensor("xT", [D, S], F16, kind="ExternalInput").ap()
    wT = nc.dram_tensor("wT", [D, 3 * D], F16, kind="ExternalInput").ap()
    bT = nc.dram_tensor("bT", [128, 24], F32, kind="ExternalInput").ap()
    bvb = nc.dram_tensor("bvb", [128, D], F32, kind="ExternalInput").ap()
    out = nc.dram_tensor("out", [S, D], F32, kind="ExternalOutput").ap()
    with tile.TileContext(nc) as tc:
        _body(tc, xT, wT, bT, bvb, out, reps=reps)
    nc.compile()
    return nc


_PROGRAM = None


def _get_program():
    global _PROGRAM
    if _PROGRAM is None:
        _PROGRAM = build_program()
    return _PROGRAM


def _prep_inputs(hidden_states, Wq, bq, Wk, bk, Wv, bv):
    hs = np.asarray(hidden_states, dtype=np.float32)
    B = hs.shape[0]
    xT = np.ascontiguousarray(hs.transpose(0, 2, 1)).astype(np.float16)
    wT = np.ascontiguousarray(np.concatenate(
        [np.asarray(Wq, dtype=np.float32).T,
         np.asarray(Wk, dtype=np.float32).T,
         np.asarray(Wv, dtype=np.float32).T], axis=1)).astype(np.float16)
    b_all = np.concatenate([np.asarray(bq, dtype=np.float32),
                            np.asarray(bk, dtype=np.float32),
                            np.asarray(bv, dtype=np.float32)])
    bT_np = np.ascontiguousarray(b_all.reshape(24, 128).T)
    bvb_np = np.ascontiguousarray(
        np.broadcast_to(np.asarray(bv, dtype=np.float32), (128, D)))
    return [{"xT": xT[b], "wT": wT, "bT": bT_np, "bvb": bvb_np}
            for b in range(B)]


def run(in_maps, **kw):
    nc = _get_program()
    return bass_utils.run_bass_kernel_spmd(
        nc, in_maps, core_ids=list(range(len(in_maps))), **kw)


def kernel(hidden_states, Wq, bq, Wk, bk, Wv, bv):
    in_maps = _prep_inputs(hidden_states, Wq, bq, Wk, bk, Wv, bv)
    res = run(in_maps)
    return np.stack([res.results[b]["out"] for b in range(len(in_maps))],
                    axis=0)
